# revision 10
# baseline (speedup 1.0000x reference)
"""MoE ExpertFeedForward (top-k routing + per-expert SwiGLU) on 8 Trainium2
NeuronCores via expert parallelism.

Host: router (logits/top-k/softmax), token dispatch, weighted combine.
Device (one expert per core): X.T laid out [D, C] column-major tokens;
  G = silu(Wg @ x), V = W1 @ x, Z = G*V, out.T = W2 @ Z,
  all as 128x128-stationary matmuls streaming token columns.
"""

import numpy as np

D_MODEL = 1024
D_FF = 4096
N_EXPERTS = 8
N_CORES = 8
P = 128
TW = 512  # token-tile width (matmul moving free dim / one PSUM bank of fp32)
DK = D_MODEL // P   # 8 contraction tiles for MM1/MM2
FI = D_FF // P      # 32 f tiles
DO = D_MODEL // P   # 8 output-d tiles for MM3

USE_BF16 = True

# ---------------------------------------------------------------------------
# BIR post-pass: TRN2 instruction encoding has ONE sync-wait slot; Tile can
# emit several waits on one instruction, which this walrus build rejects
# ("Too many sync wait commands").  Peel extra waits onto single-wait NoOps
# inserted just before the instruction on the same engine.
# ---------------------------------------------------------------------------

_bir_fix_installed = False


def _fix_bir_bytes(bir):
    import orjson

    m = orjson.loads(bir)
    changed = False
    for fn in m.get("functions", []):
        for blk in fn.get("blocks", []):
            out = []
            for inst in blk.get("instructions", []):
                si = inst.get("sync_info")
                if si:
                    waits = si.get("on_wait") or []
                    upds = si.get("on_update") or []
                    if len(waits) > 1:
                        changed = True
                        for k, w in enumerate(waits[:-1]):
                            out.append({
                                "name": f"{inst['name']}_pw{k}",
                                "opcode": "NoOp",
                                "engine": inst.get("engine", "SP"),
                                "ins": [], "outs": [],
                                "debug": inst.get("debug", 0),
                                "sync_info": {"on_wait": [w]},
                            })
                        si["on_wait"] = [waits[-1]]
                    if len(upds) > 1:
                        if inst.get("opcode") == "DMACopy":
                            raise AssertionError("multi-update DMACopy")
                        changed = True
                        extra = upds[1:]
                        si["on_update"] = [upds[0]]
                        out.append(inst)
                        for k, u in enumerate(extra):
                            out.append({
                                "name": f"{inst['name']}_pu{k}",
                                "opcode": "NoOp",
                                "engine": inst.get("engine", "SP"),
                                "ins": [], "outs": [],
                                "debug": inst.get("debug", 0),
                                "sync_info": {"on_update": [u]},
                            })
                        continue
                out.append(inst)
            blk["instructions"] = out
    return orjson.dumps(m) if changed else bir


def _install_bir_fix():
    global _bir_fix_installed
    if _bir_fix_installed:
        return
    _bir_fix_installed = True
    import concourse.bass_utils as bu
    import concourse.bass2jax as b2j

    orig = bu.compile_bir_kernel

    def patched(bir_json, tmpdir, neff_name="file.neff"):
        return orig(_fix_bir_bytes(bytes(bir_json)), tmpdir, neff_name)

    bu.compile_bir_kernel = patched
    b2j.compile_bir_kernel = patched


# ---------------------------------------------------------------------------
# Device kernel builder
# ---------------------------------------------------------------------------


def build_bass_kernel(C, repeat=1, pair=False):
    """One expert's SwiGLU FFN over C token columns (zero-padded).

    DRAM inputs (per core):
      xt   [P, DK, C]         x.T tiled:      xt[p, dk, c]   = x[c, dk*128+p]
      wg   [FI, P, DK, 128]   Wg.T tiled:     wg[fi, p, dk, fj] = Wg[fi*128+fj, dk*128+p]
      w1   [FI, P, DK, 128]   same for W1
      w2   [DO, P, FI, 128]   W2.T tiled:     w2[do, p, fi, dj] = W2[do*128+dj, fi*128+p]
    DRAM output:
      outt [P, DO, C]         out.T tiled:    outt[p, do, c] = out[c, do*128+p]

    pair=True processes two 512-token subtiles per stationary weight load,
    halving LDWEIGHTS count and weight DMA traffic.
    """
    import contextlib
    import concourse.bass as bass
    import concourse.mybir as mybir
    import concourse.tile as tile

    f32 = mybir.dt.float32
    wdt = mybir.dt.bfloat16 if USE_BF16 else f32
    odt = f32

    nc = bass.Bass("TRN2")
    xt_d = nc.dram_tensor("xt", [P, DK, C], wdt, kind="ExternalInput")
    wg_d = nc.dram_tensor("wg", [FI, P, DK, P], wdt, kind="ExternalInput")
    w1_d = nc.dram_tensor("w1", [FI, P, DK, P], wdt, kind="ExternalInput")
    w2_d = nc.dram_tensor("w2", [DO, P, FI, P], wdt, kind="ExternalInput")
    out_d = nc.dram_tensor("outt", [P, DO, C], odt, kind="ExternalOutput")

    # groups of 1-2 subtiles (each subtile <= TW wide) sharing one weight pass
    GW = 2 * TW if pair else TW
    groups = []
    g0 = 0
    while g0 < C:
        gw = min(GW, C - g0)
        subs = []
        s0 = 0
        while s0 < gw:
            sw = min(TW, gw - s0)
            subs.append((s0, sw))
            s0 += sw
        groups.append((g0, gw, subs))
        g0 += gw

    with tile.TileContext(nc) as tc:
        with (
            tc.tile_pool(name="xpool", bufs=2) as xpool,
            tc.tile_pool(name="wpool", bufs=4) as wpool,
            tc.tile_pool(name="w2pool", bufs=2) as w2pool,
            tc.tile_pool(name="zpool", bufs=1) as zpool,
            tc.tile_pool(name="gpool", bufs=4) as gpool,
            tc.tile_pool(name="opool", bufs=2) as opool,
            tc.tile_pool(name="psum", bufs=2, space="PSUM") as psum,
            tc.tile_pool(name="psumo", bufs=2, space="PSUM") as psumo,
            tc.For_i(0, repeat, 1) if repeat > 1 else contextlib.nullcontext(),
        ):
            for (g0, gw, subs) in groups:
                ns = len(subs)
                xt = xpool.tile([P, DK, GW], wdt, tag="xt")
                nc.sync.dma_start(xt[:, :, :gw], xt_d[:, :, g0:g0 + gw])
                z = zpool.tile([P, FI, GW], wdt, tag="z")
                for fi in range(FI):
                    wg = wpool.tile([P, DK, P], wdt, tag="wg")
                    nc.sync.dma_start(wg[:], wg_d[fi])
                    pgs = [psum.tile([P, TW], f32, tag=f"pg{s}", name=f"pg{s}") for s in range(ns)]
                    for dk in range(DK):
                        for s, (s0, sw) in enumerate(subs):
                            nc.tensor.matmul(
                                pgs[s][:, :sw], wg[:, dk, :],
                                xt[:, dk, s0:s0 + sw],
                                start=(dk == 0), stop=(dk == DK - 1),
                            )
                    w1 = wpool.tile([P, DK, P], wdt, tag="w1")
                    nc.sync.dma_start(w1[:], w1_d[fi])
                    pvs = [psum.tile([P, TW], f32, tag=f"pv{s}", name=f"pv{s}") for s in range(ns)]
                    for dk in range(DK):
                        for s, (s0, sw) in enumerate(subs):
                            nc.tensor.matmul(
                                pvs[s][:, :sw], w1[:, dk, :],
                                xt[:, dk, s0:s0 + sw],
                                start=(dk == 0), stop=(dk == DK - 1),
                            )
                    for s, (s0, sw) in enumerate(subs):
                        g = gpool.tile([P, TW], f32, tag="g")
                        nc.scalar.activation(
                            g[:, :sw], pgs[s][:, :sw],
                            mybir.ActivationFunctionType.Silu,
                        )
                        nc.vector.tensor_mul(
                            z[:, fi, s0:s0 + sw], g[:, :sw], pvs[s][:, :sw])
                ot = opool.tile([P, DO, GW], odt, tag="ot")
                for do in range(DO):
                    w2 = w2pool.tile([P, FI, P], wdt, tag="w2")
                    nc.sync.dma_start(w2[:], w2_d[do])
                    pos = [psumo.tile([P, TW], f32, tag=f"po{s}", name=f"po{s}") for s in range(ns)]
                    for fi in range(FI):
                        for s, (s0, sw) in enumerate(subs):
                            nc.tensor.matmul(
                                pos[s][:, :sw], w2[:, fi, :],
                                z[:, fi, s0:s0 + sw],
                                start=(fi == 0), stop=(fi == FI - 1),
                            )
                    for s, (s0, sw) in enumerate(subs):
                        nc.vector.tensor_copy(
                            ot[:, do, s0:s0 + sw], pos[s][:, :sw])
                nc.sync.dma_start(out_d[:, :, g0:g0 + gw], ot[:, :, :gw])
    return nc


# ---------------------------------------------------------------------------
# Host wrapper
# ---------------------------------------------------------------------------


def _route(xt, gate_W, gate_b, k):
    """Return per-expert (token_idx, prob) using top-k + softmax-over-top-k."""
    logits = xt @ gate_W.T + gate_b  # [T, E]
    T, E = logits.shape
    # top-k indices, matching jax.lax.top_k (descending by value)
    idx = np.argpartition(-logits, k - 1, axis=1)[:, :k]
    vals = np.take_along_axis(logits, idx, axis=1)
    order = np.argsort(-vals, axis=1, kind="stable")
    idx = np.take_along_axis(idx, order, axis=1)
    vals = np.take_along_axis(vals, order, axis=1)
    vals = vals - vals.max(axis=1, keepdims=True)
    ex = np.exp(vals)
    probs = ex / ex.sum(axis=1, keepdims=True)  # [T, k]
    per_expert = []
    flat_e = idx.reshape(-1)
    flat_t = np.repeat(np.arange(T), k)
    flat_p = probs.reshape(-1)
    for e in range(E):
        m = flat_e == e
        per_expert.append((flat_t[m], flat_p[m]))
    return per_expert


def kernel(x, gate_W, gate_b, Wg, W1, W2, num_experts_per_token):
    _install_bir_fix()
    from concourse.bass_utils import run_bass_kernel_spmd
    import ml_dtypes

    x = np.asarray(x, dtype=np.float32)
    gate_W = np.asarray(gate_W, dtype=np.float32)
    gate_b = np.asarray(gate_b, dtype=np.float32)
    Wg = np.asarray(Wg, dtype=np.float32)
    W1 = np.asarray(W1, dtype=np.float32)
    W2 = np.asarray(W2, dtype=np.float32)
    k = int(num_experts_per_token)

    B, S, D = x.shape
    T = B * S
    xt = x.reshape(T, D)
    per_expert = _route(xt, gate_W, gate_b, k)

    maxN = max(len(t) for t, _ in per_expert)
    C = max(TW, -(-maxN // P) * P)  # round up to multiple of 128, >= 512

    wdt = ml_dtypes.bfloat16 if USE_BF16 else np.float32
    nc = build_bass_kernel(C)

    in_maps = []
    for e in range(N_EXPERTS):
        tok, _ = per_expert[e]
        n = len(tok)
        xe = np.zeros((P, DK, C), dtype=wdt)
        # xt[tok].T -> [D, n] -> [DK, P, n] -> [P, DK, n]
        xe[:, :, :n] = (
            xt[tok].T.reshape(DK, P, n).transpose(1, 0, 2).astype(wdt)
        )
        wg_e = np.ascontiguousarray(
            Wg[e].reshape(FI, P, DK, P).transpose(0, 3, 2, 1)
        ).astype(wdt)
        w1_e = np.ascontiguousarray(
            W1[e].reshape(FI, P, DK, P).transpose(0, 3, 2, 1)
        ).astype(wdt)
        w2_e = np.ascontiguousarray(
            W2[e].reshape(DO, P, FI, P).transpose(0, 3, 2, 1)
        ).astype(wdt)
        in_maps.append({"xt": xe, "wg": wg_e, "w1": w1_e, "w2": w2_e})

    res = run_bass_kernel_spmd(nc, in_maps, core_ids=list(range(N_CORES)))

    out = np.zeros((T, D), dtype=np.float32)
    for e in range(N_EXPERTS):
        tok, prob = per_expert[e]
        n = len(tok)
        oe = np.asarray(res.results[e]["outt"], dtype=np.float32)  # [P, DO, C]
        oe = oe[:, :, :n].transpose(1, 0, 2).reshape(D, n).T  # [n, D]
        out[tok] += oe * prob[:, None].astype(np.float32)
    return out.reshape(B, S, D)


# revision 13
# speedup vs baseline: 1.1175x; 1.1175x over previous
"""MoE ExpertFeedForward (top-k routing + per-expert SwiGLU) on 8 Trainium2
NeuronCores via expert parallelism.

Host: router (logits/top-k/softmax), token dispatch, weighted combine.
Device (one expert per core): X.T laid out [D, C] column-major tokens;
  G = silu(Wg @ x), V = W1 @ x, Z = G*V, out.T = W2 @ Z,
  all as 128x128-stationary matmuls streaming token columns.
"""

import numpy as np

D_MODEL = 1024
D_FF = 4096
N_EXPERTS = 8
N_CORES = 8
P = 128
TW = 512  # token-tile width (matmul moving free dim / one PSUM bank of fp32)
DK = D_MODEL // P   # 8 contraction tiles for MM1/MM2
FI = D_FF // P      # 32 f tiles
DO = D_MODEL // P   # 8 output-d tiles for MM3

USE_BF16 = True

# ---------------------------------------------------------------------------
# BIR post-pass: TRN2 instruction encoding has ONE sync-wait slot; Tile can
# emit several waits on one instruction, which this walrus build rejects
# ("Too many sync wait commands").  Peel extra waits onto single-wait NoOps
# inserted just before the instruction on the same engine.
# ---------------------------------------------------------------------------

_bir_fix_installed = False


def _ldw_key(inst):
    import orjson
    return orjson.dumps([
        inst.get("ins"), inst.get("tile_position"), inst.get("tile_size"),
        inst.get("perf_mode"), inst.get("is_transpose"),
    ])


def _dedup_ldweights(m):
    """Remove Ldweights identical to the previous Ldweights on PE (weights
    persist in the array across non-self-loading Matmults). Waits on removed
    instructions are kept on a NoOp."""
    changed = False
    for fn in m.get("functions", []):
        for blk in fn.get("blocks", []):
            out = []
            last_ldw_key = None
            for inst in blk.get("instructions", []):
                op = inst.get("opcode")
                if inst.get("engine") == "PE":
                    if op == "Ldweights":
                        key = _ldw_key(inst)
                        if key == last_ldw_key:
                            changed = True
                            si = inst.get("sync_info") or {}
                            if si.get("on_wait") or si.get("on_update"):
                                out.append({
                                    "name": f"{inst['name']}_dw",
                                    "opcode": "NoOp",
                                    "engine": "PE",
                                    "ins": [], "outs": [],
                                    "debug": inst.get("debug", 0),
                                    "sync_info": si,
                                })
                            continue
                        last_ldw_key = key
                    elif op == "Matmult":
                        if inst.get("ldweights", True):
                            last_ldw_key = None
                    else:
                        # branches, drains, transposes etc: be conservative
                        if op not in ("NoOp", "EventSemaphore"):
                            last_ldw_key = None
                out.append(inst)
            blk["instructions"] = out
    return changed


def _fix_bir_bytes(bir):
    import orjson

    m = orjson.loads(bir)
    changed = _dedup_ldweights(m)
    for fn in m.get("functions", []):
        for blk in fn.get("blocks", []):
            out = []
            for inst in blk.get("instructions", []):
                si = inst.get("sync_info")
                if si:
                    waits = si.get("on_wait") or []
                    upds = si.get("on_update") or []
                    if len(waits) > 1:
                        changed = True
                        for k, w in enumerate(waits[:-1]):
                            out.append({
                                "name": f"{inst['name']}_pw{k}",
                                "opcode": "NoOp",
                                "engine": inst.get("engine", "SP"),
                                "ins": [], "outs": [],
                                "debug": inst.get("debug", 0),
                                "sync_info": {"on_wait": [w]},
                            })
                        si["on_wait"] = [waits[-1]]
                    if len(upds) > 1:
                        if inst.get("opcode") == "DMACopy":
                            raise AssertionError("multi-update DMACopy")
                        changed = True
                        extra = upds[1:]
                        si["on_update"] = [upds[0]]
                        out.append(inst)
                        for k, u in enumerate(extra):
                            out.append({
                                "name": f"{inst['name']}_pu{k}",
                                "opcode": "NoOp",
                                "engine": inst.get("engine", "SP"),
                                "ins": [], "outs": [],
                                "debug": inst.get("debug", 0),
                                "sync_info": {"on_update": [u]},
                            })
                        continue
                out.append(inst)
            blk["instructions"] = out
    return orjson.dumps(m) if changed else bir


def _install_bir_fix():
    global _bir_fix_installed
    if _bir_fix_installed:
        return
    _bir_fix_installed = True
    import concourse.bass_utils as bu
    import concourse.bass2jax as b2j

    orig = bu.compile_bir_kernel

    def patched(bir_json, tmpdir, neff_name="file.neff"):
        return orig(_fix_bir_bytes(bytes(bir_json)), tmpdir, neff_name)

    bu.compile_bir_kernel = patched
    b2j.compile_bir_kernel = patched


# ---------------------------------------------------------------------------
# Device kernel builder
# ---------------------------------------------------------------------------


def build_bass_kernel(C, repeat=1, pair=False):
    """One expert's SwiGLU FFN over C token columns (zero-padded).

    DRAM inputs (per core):
      xt   [P, DK, C]         x.T tiled:      xt[p, dk, c]   = x[c, dk*128+p]
      wg   [FI, P, DK, 128]   Wg.T tiled:     wg[fi, p, dk, fj] = Wg[fi*128+fj, dk*128+p]
      w1   [FI, P, DK, 128]   same for W1
      w2   [DO, P, FI, 128]   W2.T tiled:     w2[do, p, fi, dj] = W2[do*128+dj, fi*128+p]
    DRAM output:
      outt [P, DO, C]         out.T tiled:    outt[p, do, c] = out[c, do*128+p]

    pair=True processes two 512-token subtiles per stationary weight load,
    halving LDWEIGHTS count and weight DMA traffic.
    """
    import contextlib
    import concourse.bass as bass
    import concourse.mybir as mybir
    import concourse.tile as tile

    f32 = mybir.dt.float32
    wdt = mybir.dt.bfloat16 if USE_BF16 else f32
    odt = f32

    nc = bass.Bass("TRN2")
    xt_d = nc.dram_tensor("xt", [P, DK, C], wdt, kind="ExternalInput")
    wg_d = nc.dram_tensor("wg", [FI, P, DK, P], wdt, kind="ExternalInput")
    w1_d = nc.dram_tensor("w1", [FI, P, DK, P], wdt, kind="ExternalInput")
    w2_d = nc.dram_tensor("w2", [DO, P, FI, P], wdt, kind="ExternalInput")
    out_d = nc.dram_tensor("outt", [P, DO, C], odt, kind="ExternalOutput")

    # groups of 1-2 subtiles (each subtile <= TW wide) sharing one weight pass
    GW = 2 * TW if pair else TW
    groups = []
    g0 = 0
    while g0 < C:
        gw = min(GW, C - g0)
        subs = []
        s0 = 0
        while s0 < gw:
            sw = min(TW, gw - s0)
            subs.append((s0, sw))
            s0 += sw
        groups.append((g0, gw, subs))
        g0 += gw

    with tile.TileContext(nc) as tc:
        with (
            tc.tile_pool(name="xpool", bufs=2) as xpool,
            tc.tile_pool(name="wpool", bufs=4) as wpool,
            tc.tile_pool(name="w2pool", bufs=2) as w2pool,
            tc.tile_pool(name="zpool", bufs=1) as zpool,
            tc.tile_pool(name="gpool", bufs=4) as gpool,
            tc.tile_pool(name="opool", bufs=2) as opool,
            tc.tile_pool(name="psum", bufs=(1 if pair else 2), space="PSUM") as psum,
            tc.tile_pool(name="psumo", bufs=2, space="PSUM") as psumo,
            tc.For_i(0, repeat, 1) if repeat > 1 else contextlib.nullcontext(),
        ):
            for (g0, gw, subs) in groups:
                ns = len(subs)
                xt = xpool.tile([P, DK, GW], wdt, tag="xt")
                nc.sync.dma_start(xt[:, :, :gw], xt_d[:, :, g0:g0 + gw])
                z = zpool.tile([P, FI, GW], wdt, tag="z")
                for fi in range(FI):
                    wg = wpool.tile([P, DK, P], wdt, tag="wg")
                    nc.sync.dma_start(wg[:], wg_d[fi])
                    pgs = [psum.tile([P, TW], f32, tag=f"pg{s}", name=f"pg{s}") for s in range(ns)]
                    for dk in range(DK):
                        for s, (s0, sw) in enumerate(subs):
                            nc.tensor.matmul(
                                pgs[s][:, :sw], wg[:, dk, :],
                                xt[:, dk, s0:s0 + sw],
                                start=(dk == 0), stop=(dk == DK - 1),
                            )
                    w1 = wpool.tile([P, DK, P], wdt, tag="w1")
                    nc.sync.dma_start(w1[:], w1_d[fi])
                    pvs = [psum.tile([P, TW], f32, tag=f"pv{s}", name=f"pv{s}") for s in range(ns)]
                    for dk in range(DK):
                        for s, (s0, sw) in enumerate(subs):
                            nc.tensor.matmul(
                                pvs[s][:, :sw], w1[:, dk, :],
                                xt[:, dk, s0:s0 + sw],
                                start=(dk == 0), stop=(dk == DK - 1),
                            )
                    for s, (s0, sw) in enumerate(subs):
                        g = gpool.tile([P, TW], f32, tag="g")
                        nc.scalar.activation(
                            g[:, :sw], pgs[s][:, :sw],
                            mybir.ActivationFunctionType.Silu,
                        )
                        nc.vector.tensor_mul(
                            z[:, fi, s0:s0 + sw], g[:, :sw], pvs[s][:, :sw])
                ot = opool.tile([P, DO, GW], odt, tag="ot")
                for do in range(DO):
                    w2 = w2pool.tile([P, FI, P], wdt, tag="w2")
                    nc.sync.dma_start(w2[:], w2_d[do])
                    pos = [psumo.tile([P, TW], f32, tag=f"po{s}", name=f"po{s}") for s in range(ns)]
                    for fi in range(FI):
                        for s, (s0, sw) in enumerate(subs):
                            nc.tensor.matmul(
                                pos[s][:, :sw], w2[:, fi, :],
                                z[:, fi, s0:s0 + sw],
                                start=(fi == 0), stop=(fi == FI - 1),
                            )
                    for s, (s0, sw) in enumerate(subs):
                        nc.vector.tensor_copy(
                            ot[:, do, s0:s0 + sw], pos[s][:, :sw])
                nc.sync.dma_start(out_d[:, :, g0:g0 + gw], ot[:, :, :gw])
    return nc


# ---------------------------------------------------------------------------
# Host wrapper
# ---------------------------------------------------------------------------


def _route(xt, gate_W, gate_b, k):
    """Return per-expert (token_idx, prob) using top-k + softmax-over-top-k."""
    logits = xt @ gate_W.T + gate_b  # [T, E]
    T, E = logits.shape
    # top-k indices, matching jax.lax.top_k (descending by value)
    idx = np.argpartition(-logits, k - 1, axis=1)[:, :k]
    vals = np.take_along_axis(logits, idx, axis=1)
    order = np.argsort(-vals, axis=1, kind="stable")
    idx = np.take_along_axis(idx, order, axis=1)
    vals = np.take_along_axis(vals, order, axis=1)
    vals = vals - vals.max(axis=1, keepdims=True)
    ex = np.exp(vals)
    probs = ex / ex.sum(axis=1, keepdims=True)  # [T, k]
    per_expert = []
    flat_e = idx.reshape(-1)
    flat_t = np.repeat(np.arange(T), k)
    flat_p = probs.reshape(-1)
    for e in range(E):
        m = flat_e == e
        per_expert.append((flat_t[m], flat_p[m]))
    return per_expert


def kernel(x, gate_W, gate_b, Wg, W1, W2, num_experts_per_token):
    _install_bir_fix()
    from concourse.bass_utils import run_bass_kernel_spmd
    import ml_dtypes

    x = np.asarray(x, dtype=np.float32)
    gate_W = np.asarray(gate_W, dtype=np.float32)
    gate_b = np.asarray(gate_b, dtype=np.float32)
    Wg = np.asarray(Wg, dtype=np.float32)
    W1 = np.asarray(W1, dtype=np.float32)
    W2 = np.asarray(W2, dtype=np.float32)
    k = int(num_experts_per_token)

    B, S, D = x.shape
    T = B * S
    xt = x.reshape(T, D)
    per_expert = _route(xt, gate_W, gate_b, k)

    maxN = max(len(t) for t, _ in per_expert)
    C = max(TW, -(-maxN // P) * P)  # round up to multiple of 128, >= 512

    wdt = ml_dtypes.bfloat16 if USE_BF16 else np.float32
    nc = build_bass_kernel(C)

    in_maps = []
    for e in range(N_EXPERTS):
        tok, _ = per_expert[e]
        n = len(tok)
        xe = np.zeros((P, DK, C), dtype=wdt)
        # xt[tok].T -> [D, n] -> [DK, P, n] -> [P, DK, n]
        xe[:, :, :n] = (
            xt[tok].T.reshape(DK, P, n).transpose(1, 0, 2).astype(wdt)
        )
        wg_e = np.ascontiguousarray(
            Wg[e].reshape(FI, P, DK, P).transpose(0, 3, 2, 1)
        ).astype(wdt)
        w1_e = np.ascontiguousarray(
            W1[e].reshape(FI, P, DK, P).transpose(0, 3, 2, 1)
        ).astype(wdt)
        w2_e = np.ascontiguousarray(
            W2[e].reshape(DO, P, FI, P).transpose(0, 3, 2, 1)
        ).astype(wdt)
        in_maps.append({"xt": xe, "wg": wg_e, "w1": w1_e, "w2": w2_e})

    # The axon terminal occasionally wedges (NRT_EXEC_UNIT_UNRECOVERABLE);
    # a backend reset + retry usually recovers.
    try:
        res = run_bass_kernel_spmd(nc, in_maps, core_ids=list(range(N_CORES)))
    except Exception:
        import time as _time
        import jax as _jax
        _time.sleep(20)
        try:
            _jax.clear_caches()
            _jax.extend.backend.clear_backends()
        except Exception:
            pass
        res = run_bass_kernel_spmd(nc, in_maps, core_ids=list(range(N_CORES)))

    out = np.zeros((T, D), dtype=np.float32)
    for e in range(N_EXPERTS):
        tok, prob = per_expert[e]
        n = len(tok)
        oe = np.asarray(res.results[e]["outt"], dtype=np.float32)  # [P, DO, C]
        oe = oe[:, :, :n].transpose(1, 0, 2).reshape(D, n).T  # [n, D]
        out[tok] += oe * prob[:, None].astype(np.float32)
    return out.reshape(B, S, D)


# revision 18
# speedup vs baseline: 1.1985x; 1.0725x over previous
"""MoE ExpertFeedForward (top-k routing + per-expert SwiGLU) on 8 Trainium2
NeuronCores via expert parallelism.

Host: router (logits/top-k/softmax), token dispatch, weighted combine.
Device (one expert per core): X.T laid out [D, C] column-major tokens;
  G = silu(Wg @ x), V = W1 @ x, Z = G*V, out.T = W2 @ Z,
  all as 128x128-stationary matmuls streaming token columns.
"""

import numpy as np

D_MODEL = 1024
D_FF = 4096
N_EXPERTS = 8
N_CORES = 8
P = 128
TW = 512  # token-tile width (matmul moving free dim / one PSUM bank of fp32)
DK = D_MODEL // P   # 8 contraction tiles for MM1/MM2
FI = D_FF // P      # 32 f tiles
DO = D_MODEL // P   # 8 output-d tiles for MM3

USE_BF16 = True

# ---------------------------------------------------------------------------
# BIR post-pass: TRN2 instruction encoding has ONE sync-wait slot; Tile can
# emit several waits on one instruction, which this walrus build rejects
# ("Too many sync wait commands").  Peel extra waits onto single-wait NoOps
# inserted just before the instruction on the same engine.
# ---------------------------------------------------------------------------

_bir_fix_installed = False


def _ldw_key(inst):
    import orjson
    return orjson.dumps([
        inst.get("ins"), inst.get("tile_position"), inst.get("tile_size"),
        inst.get("perf_mode"), inst.get("is_transpose"),
    ])


def _dedup_ldweights(m):
    """Remove Ldweights identical to the previous Ldweights on PE (weights
    persist in the array across non-self-loading Matmults). Waits on removed
    instructions are kept on a NoOp."""
    changed = False
    for fn in m.get("functions", []):
        for blk in fn.get("blocks", []):
            out = []
            last_ldw_key = None
            for inst in blk.get("instructions", []):
                op = inst.get("opcode")
                if inst.get("engine") == "PE":
                    if op == "Ldweights":
                        key = _ldw_key(inst)
                        if key == last_ldw_key:
                            changed = True
                            si = inst.get("sync_info") or {}
                            if si.get("on_wait") or si.get("on_update"):
                                out.append({
                                    "name": f"{inst['name']}_dw",
                                    "opcode": "NoOp",
                                    "engine": "PE",
                                    "ins": [], "outs": [],
                                    "debug": inst.get("debug", 0),
                                    "sync_info": si,
                                })
                            continue
                        last_ldw_key = key
                    elif op == "Matmult":
                        if inst.get("ldweights", True):
                            last_ldw_key = None
                    else:
                        # branches, drains, transposes etc: be conservative
                        if op not in ("NoOp", "EventSemaphore"):
                            last_ldw_key = None
                out.append(inst)
            blk["instructions"] = out
    return changed


def _fix_bir_bytes(bir):
    import orjson

    m = orjson.loads(bir)
    changed = _dedup_ldweights(m)
    for fn in m.get("functions", []):
        for blk in fn.get("blocks", []):
            out = []
            for inst in blk.get("instructions", []):
                si = inst.get("sync_info")
                if si:
                    waits = si.get("on_wait") or []
                    upds = si.get("on_update") or []
                    if len(waits) > 1:
                        changed = True
                        for k, w in enumerate(waits[:-1]):
                            out.append({
                                "name": f"{inst['name']}_pw{k}",
                                "opcode": "NoOp",
                                "engine": inst.get("engine", "SP"),
                                "ins": [], "outs": [],
                                "debug": inst.get("debug", 0),
                                "sync_info": {"on_wait": [w]},
                            })
                        si["on_wait"] = [waits[-1]]
                    if len(upds) > 1:
                        if inst.get("opcode") == "DMACopy":
                            raise AssertionError("multi-update DMACopy")
                        changed = True
                        extra = upds[1:]
                        si["on_update"] = [upds[0]]
                        out.append(inst)
                        for k, u in enumerate(extra):
                            out.append({
                                "name": f"{inst['name']}_pu{k}",
                                "opcode": "NoOp",
                                "engine": inst.get("engine", "SP"),
                                "ins": [], "outs": [],
                                "debug": inst.get("debug", 0),
                                "sync_info": {"on_update": [u]},
                            })
                        continue
                out.append(inst)
            blk["instructions"] = out
    return orjson.dumps(m) if changed else bir


def _install_bir_fix():
    global _bir_fix_installed
    if _bir_fix_installed:
        return
    _bir_fix_installed = True
    import concourse.bass_utils as bu
    import concourse.bass2jax as b2j

    orig = bu.compile_bir_kernel

    def patched(bir_json, tmpdir, neff_name="file.neff"):
        return orig(_fix_bir_bytes(bytes(bir_json)), tmpdir, neff_name)

    bu.compile_bir_kernel = patched
    b2j.compile_bir_kernel = patched


# ---------------------------------------------------------------------------
# Device kernel builder
# ---------------------------------------------------------------------------


def build_bass_kernel(C, repeat=1, pair=False, bufs=None):
    """One expert's SwiGLU FFN over C token columns (zero-padded).

    DRAM inputs (per core):
      xt   [P, DK, C]         x.T tiled:      xt[p, dk, c]   = x[c, dk*128+p]
      wg   [FI, P, DK, 128]   Wg.T tiled:     wg[fi, p, dk, fj] = Wg[fi*128+fj, dk*128+p]
      w1   [FI, P, DK, 128]   same for W1
      w2   [DO, P, FI, 128]   W2.T tiled:     w2[do, p, fi, dj] = W2[do*128+dj, fi*128+p]
    DRAM output:
      outt [P, DO, C]         out.T tiled:    outt[p, do, c] = out[c, do*128+p]

    pair=True processes two 512-token subtiles per stationary weight load,
    halving LDWEIGHTS count and weight DMA traffic.
    """
    import contextlib
    import concourse.bass as bass
    import concourse.mybir as mybir
    import concourse.tile as tile

    bd = {"x": 2, "w": 6, "w2": 6, "z": 1, "g": 4, "o": 2,
          "ps": (1 if pair else 2), "pso": 2}
    if bufs:
        bd.update(bufs)

    f32 = mybir.dt.float32
    wdt = mybir.dt.bfloat16 if USE_BF16 else f32
    odt = f32

    nc = bass.Bass("TRN2")
    xt_d = nc.dram_tensor("xt", [P, DK, C], wdt, kind="ExternalInput")
    wg_d = nc.dram_tensor("wg", [FI, P, DK, P], wdt, kind="ExternalInput")
    w1_d = nc.dram_tensor("w1", [FI, P, DK, P], wdt, kind="ExternalInput")
    w2_d = nc.dram_tensor("w2", [DO, P, FI, P], wdt, kind="ExternalInput")
    out_d = nc.dram_tensor("outt", [P, DO, C], odt, kind="ExternalOutput")

    # groups of 1-2 subtiles (each subtile <= TW wide) sharing one weight pass
    GW = 2 * TW if pair else TW
    groups = []
    g0 = 0
    while g0 < C:
        gw = min(GW, C - g0)
        subs = []
        s0 = 0
        while s0 < gw:
            sw = min(TW, gw - s0)
            subs.append((s0, sw))
            s0 += sw
        groups.append((g0, gw, subs))
        g0 += gw

    with tile.TileContext(nc) as tc:
        with (
            tc.tile_pool(name="xpool", bufs=bd["x"]) as xpool,
            tc.tile_pool(name="wpool", bufs=bd["w"]) as wpool,
            tc.tile_pool(name="w2pool", bufs=bd["w2"]) as w2pool,
            tc.tile_pool(name="zpool", bufs=bd["z"]) as zpool,
            tc.tile_pool(name="gpool", bufs=bd["g"]) as gpool,
            tc.tile_pool(name="opool", bufs=bd["o"]) as opool,
            tc.tile_pool(name="psum", bufs=bd["ps"], space="PSUM") as psum,
            tc.tile_pool(name="psumo", bufs=bd["pso"], space="PSUM") as psumo,
            tc.For_i(0, repeat, 1) if repeat > 1 else contextlib.nullcontext(),
        ):
            for (g0, gw, subs) in groups:
                ns = len(subs)
                xt = xpool.tile([P, DK, GW], wdt, tag="xt")
                nc.sync.dma_start(xt[:, :, :gw], xt_d[:, :, g0:g0 + gw])
                z = zpool.tile([P, FI, GW], wdt, tag="z")
                for fi in range(FI):
                    wg = wpool.tile([P, DK, P], wdt, tag="wg")
                    nc.sync.dma_start(wg[:], wg_d[fi])
                    w1 = wpool.tile([P, DK, P], wdt, tag="w1")
                    nc.sync.dma_start(w1[:], w1_d[fi])
                    pgs = [psum.tile([P, TW], f32, tag=f"pg{s}", name=f"pg{s}") for s in range(ns)]
                    pvs = [psum.tile([P, TW], f32, tag=f"pv{s}", name=f"pv{s}") for s in range(ns)]
                    for dk in range(DK):
                        for s, (s0, sw) in enumerate(subs):
                            nc.tensor.matmul(
                                pgs[s][:, :sw], wg[:, dk, :],
                                xt[:, dk, s0:s0 + sw],
                                start=(dk == 0), stop=(dk == DK - 1),
                            )
                            nc.tensor.matmul(
                                pvs[s][:, :sw], w1[:, dk, :],
                                xt[:, dk, s0:s0 + sw],
                                start=(dk == 0), stop=(dk == DK - 1),
                            )
                    for s, (s0, sw) in enumerate(subs):
                        g = gpool.tile([P, TW], f32, tag="g")
                        nc.scalar.activation(
                            g[:, :sw], pgs[s][:, :sw],
                            mybir.ActivationFunctionType.Silu,
                        )
                        nc.vector.tensor_mul(
                            z[:, fi, s0:s0 + sw], g[:, :sw], pvs[s][:, :sw])
                ot = opool.tile([P, DO, GW], odt, tag="ot")
                for do in range(DO):
                    w2a = w2pool.tile([P, FI // 2, P], wdt, tag="w2a")
                    nc.sync.dma_start(w2a[:], w2_d[do, :, :FI // 2])
                    w2b = w2pool.tile([P, FI // 2, P], wdt, tag="w2b")
                    nc.sync.dma_start(w2b[:], w2_d[do, :, FI // 2:])
                    pos = [psumo.tile([P, TW], f32, tag=f"po{s}", name=f"po{s}") for s in range(ns)]
                    for fi in range(FI):
                        w2h = w2a if fi < FI // 2 else w2b
                        j = fi if fi < FI // 2 else fi - FI // 2
                        for s, (s0, sw) in enumerate(subs):
                            nc.tensor.matmul(
                                pos[s][:, :sw], w2h[:, j, :],
                                z[:, fi, s0:s0 + sw],
                                start=(fi == 0), stop=(fi == FI - 1),
                            )
                    for s, (s0, sw) in enumerate(subs):
                        nc.vector.tensor_copy(
                            ot[:, do, s0:s0 + sw], pos[s][:, :sw])
                    nc.sync.dma_start(out_d[:, do, g0:g0 + gw], ot[:, do, :gw])
    return nc


# ---------------------------------------------------------------------------
# Host wrapper
# ---------------------------------------------------------------------------


def _route(xt, gate_W, gate_b, k):
    """Return per-expert (token_idx, prob) using top-k + softmax-over-top-k."""
    logits = xt @ gate_W.T + gate_b  # [T, E]
    T, E = logits.shape
    # top-k indices, matching jax.lax.top_k (descending by value)
    idx = np.argpartition(-logits, k - 1, axis=1)[:, :k]
    vals = np.take_along_axis(logits, idx, axis=1)
    order = np.argsort(-vals, axis=1, kind="stable")
    idx = np.take_along_axis(idx, order, axis=1)
    vals = np.take_along_axis(vals, order, axis=1)
    vals = vals - vals.max(axis=1, keepdims=True)
    ex = np.exp(vals)
    probs = ex / ex.sum(axis=1, keepdims=True)  # [T, k]
    per_expert = []
    flat_e = idx.reshape(-1)
    flat_t = np.repeat(np.arange(T), k)
    flat_p = probs.reshape(-1)
    for e in range(E):
        m = flat_e == e
        per_expert.append((flat_t[m], flat_p[m]))
    return per_expert


def kernel(x, gate_W, gate_b, Wg, W1, W2, num_experts_per_token):
    _install_bir_fix()
    from concourse.bass_utils import run_bass_kernel_spmd
    import ml_dtypes

    x = np.asarray(x, dtype=np.float32)
    gate_W = np.asarray(gate_W, dtype=np.float32)
    gate_b = np.asarray(gate_b, dtype=np.float32)
    Wg = np.asarray(Wg, dtype=np.float32)
    W1 = np.asarray(W1, dtype=np.float32)
    W2 = np.asarray(W2, dtype=np.float32)
    k = int(num_experts_per_token)

    B, S, D = x.shape
    T = B * S
    xt = x.reshape(T, D)
    per_expert = _route(xt, gate_W, gate_b, k)

    maxN = max(len(t) for t, _ in per_expert)
    C = max(TW, -(-maxN // P) * P)  # round up to multiple of 128, >= 512

    wdt = ml_dtypes.bfloat16 if USE_BF16 else np.float32
    nc = build_bass_kernel(C)

    in_maps = []
    for e in range(N_EXPERTS):
        tok, _ = per_expert[e]
        n = len(tok)
        xe = np.zeros((P, DK, C), dtype=wdt)
        # xt[tok].T -> [D, n] -> [DK, P, n] -> [P, DK, n]
        xe[:, :, :n] = (
            xt[tok].T.reshape(DK, P, n).transpose(1, 0, 2).astype(wdt)
        )
        wg_e = np.ascontiguousarray(
            Wg[e].reshape(FI, P, DK, P).transpose(0, 3, 2, 1)
        ).astype(wdt)
        w1_e = np.ascontiguousarray(
            W1[e].reshape(FI, P, DK, P).transpose(0, 3, 2, 1)
        ).astype(wdt)
        w2_e = np.ascontiguousarray(
            W2[e].reshape(DO, P, FI, P).transpose(0, 3, 2, 1)
        ).astype(wdt)
        in_maps.append({"xt": xe, "wg": wg_e, "w1": w1_e, "w2": w2_e})

    # The axon terminal occasionally wedges (NRT_EXEC_UNIT_UNRECOVERABLE);
    # a backend reset + retry after a backoff usually recovers.
    res = None
    for attempt, backoff in ((0, 0), (1, 30), (2, 90)):
        if attempt:
            import time as _time
            import jax as _jax
            _time.sleep(backoff)
            try:
                _jax.clear_caches()
                _jax.extend.backend.clear_backends()
            except Exception:
                pass
        try:
            res = run_bass_kernel_spmd(nc, in_maps, core_ids=list(range(N_CORES)))
            break
        except Exception:
            if attempt == 2:
                raise

    out = np.zeros((T, D), dtype=np.float32)
    for e in range(N_EXPERTS):
        tok, prob = per_expert[e]
        n = len(tok)
        oe = np.asarray(res.results[e]["outt"], dtype=np.float32)  # [P, DO, C]
        oe = oe[:, :, :n].transpose(1, 0, 2).reshape(D, n).T  # [n, D]
        out[tok] += oe * prob[:, None].astype(np.float32)
    return out.reshape(B, S, D)
